# revision 14
# baseline (speedup 1.0000x reference)
"""CTAttention Trainium2 kernel (v2).

Full-input contract: kernel(**inputs) takes the unsharded numpy inputs and
returns the full [total, C] output. Internally: data-parallel over the batch
axis B=8 across 8 NeuronCores (attention is independent per batch element);
qkv/proj weights replicated; ragged scatter/gather bookkeeping on the host.

Per-core dataflow (batch b, dense 1024 windows, 8 heads, head_dim 32):
  X^T[128,2048] -> Q^T/K^T (f32r, channel-on-partition) and V' (bf16,
  [kpos, j, head, 1+32] with an all-ones column 0).
  Attention runs in 4 phases (query-chunk c outer, head-group grp inner),
  8 key-block steps each, software-pipelined at exp cadence:
    scores: S^T = per-head K=32 matmuls, 4-way row-quadrant-packed (f32r),
            two [128,1024] psums per step
    exp:    one ScalarE Exp per psum with the key-padding mask as a
            per-partition bias (masked scores underflow to exactly 0);
            output P^T in bf16. ScalarE runs ONLY these 64 exps - it is
            the bottleneck engine (~71us of 8.4M exp elements).
    PV:     bf16 matmuls with M=33 (ones column first): row 0 of each
            33-row strip accumulates the softmax denominator for free -
            no separate rowsum matmuls.
  Normalization per phase: DVE reciprocal reads the denominator psum rows
  {0,64} directly, a [128,128] f32r selector matmul broadcasts 1/denom to
  the O rows, one DVE multiply per head-pair writes normalized O into a
  block-permuted otf; the output projection uses a host-packed zero-padded
  pwx whose rows match the permuted layout. Projection + output DMA per
  query chunk overlap the next phase; only chunk c1's chain is tail-serial.

Exact algebraic simplifications vs the reference:
  - K bias dropped (softmax is invariant to per-query constant shifts)
  - V bias folded into the proj bias (softmax weights sum to 1)
  - head-dim scale folded into the exp's input scale

Environment workarounds (this walrus build): at most one sem wait per
instruction (waits hoisted onto injected NOPs), fp32/fp32r matmuls require
dst partition base 0, no gpsimd extended instructions, no custom DVE ops.
"""

import sys

if "/opt/trn_rl_repo" not in sys.path:
    sys.path.insert(0, "/opt/trn_rl_repo")

import numpy as np

B = 8
C = 256
H = 8
HD = 32
MAXW = 1024
SCALE = HD ** -0.5
NEG_THRESH = -1e8  # mask values below this count as fully masked

_cached = {}


def _build_nc():
    import bass_rust
    import concourse.bass as bass
    import concourse.tile as tile
    import concourse.mybir as mybir
    from concourse.vector_clock import ScopedClock

    # ---- workaround: this walrus build accepts at most ONE sem wait per
    # instruction ("Too many sync wait commands" in setupSyncWait). Tile
    # attaches multi-sem waits freely. Split: hoist all but the last wait of
    # every committed instruction onto injected same-engine NOPs, and split
    # the final drain the same way.
    _ctr = [0]

    def _hoist_excess_waits(tc_self, inst, orig_add):
        si = inst.sync_info
        if si is not None:
            waits = list(si.on_wait or [])
            if len(waits) > 1:
                for w in waits[:-1]:
                    _ctr[0] += 1
                    nop = mybir.InstNoOp(name=f"waitsplit-{_ctr[0]}")
                    nop.engine = inst.engine
                    nop.sync_info = bass_rust.SyncInfo(on_wait=[w], on_update=[])
                    orig_add(tc_self, nop)
                si.on_wait = waits[-1:]
        orig_add(tc_self, inst)

    if not getattr(tile.TileContext, "_waitsplit_patched", False):
        _orig_add_instruction = tile.TileContext._add_instruction

        def _split_add_instruction(self, inst):
            _hoist_excess_waits(self, inst, _orig_add_instruction)

        tile.TileContext._add_instruction = _split_add_instruction

        def _patched_drain_and_barrier(self, tick_clock, wait_clock):
            nc = self.nc
            d0 = nc.sync.drain()
            wait_clock.add_sem_waits(
                d0.ins, ScopedClock({None: tick_clock.global_clock})
            )
            si = d0.ins.sync_info
            waits = list(si.on_wait) if si is not None else []
            if len(waits) > 1:
                si.on_wait = waits[0:1]
                for w in waits[1:]:
                    dk = nc.sync.drain()
                    dk.ins.sync_info = bass_rust.SyncInfo(on_wait=[w], on_update=[])
            nc.all_engine_barrier()
            assert self.sems is not None
            popped = nc._tile_sem_poison_stack.pop()
            assert popped is self._sem_poison
            nc.clear_and_free_semaphores(list(self.sems.allocated().values()))
            nc.all_engine_barrier()

        tile.TileContext._drain_and_barrier = _patched_drain_and_barrier
        tile.TileContext._waitsplit_patched = True

    dt = mybir.dt
    f32 = dt.float32
    f32r = dt.float32r
    bf16 = dt.bfloat16
    AF = mybir.ActivationFunctionType

    nc = bass.Bass(
        "TRN2",
        target_bir_lowering=False,
        debug=False,
        num_devices=1,
        enable_asserts=False,
    )

    xt_d = nc.dram_tensor("xt", [128, 2048], f32r, kind="ExternalInput").ap()
    qw_d = nc.dram_tensor("qw", [128, 1536], f32r, kind="ExternalInput").ap()
    qb_d = nc.dram_tensor("qb", [128, 2], f32, kind="ExternalInput").ap()
    pwx_d = nc.dram_tensor("pwx", [128, 1024], f32r, kind="ExternalInput").ap()
    pb_d = nc.dram_tensor("pb", [128, 2], f32, kind="ExternalInput").ap()
    mask_d = nc.dram_tensor("mask", [128, 8], f32, kind="ExternalInput").ap()
    sel_d = nc.dram_tensor("sel", [128, 128], f32r, kind="ExternalInput").ap()
    vones_d = nc.dram_tensor("vones", [128, 2176], dt.bfloat16, kind="ExternalInput").ap()
    yt_d = nc.dram_tensor("yt", [128, 2048], f32, kind="ExternalOutput").ap()

    with tile.TileContext(nc) as tc:
        with (
            tc.tile_pool(name="const", bufs=1) as const_pool,
            tc.tile_pool(name="big", bufs=1) as big_pool,
            tc.tile_pool(name="pt", bufs=4) as pt_pool,
            tc.tile_pool(name="ps_s4a", bufs=1, space="PSUM") as ps_s4a,
            tc.tile_pool(name="ps_s4b", bufs=1, space="PSUM") as ps_s4b,
            tc.tile_pool(name="ps_ov", bufs=1, space="PSUM") as ps_ov,
            tc.tile_pool(name="ps_bp", bufs=1, space="PSUM") as ps_bp,
        ):
            xt = const_pool.tile([128, 2048], f32r, tag="xt")
            qw = const_pool.tile([128, 1536], f32r, tag="qw")
            qb = const_pool.tile([128, 2], f32, tag="qb")
            pwx = const_pool.tile([128, 1024], f32r, tag="pwx")
            pb = const_pool.tile([128, 2], f32, tag="pb")
            mask = const_pool.tile([128, 8], f32, tag="mask")
            sel = const_pool.tile([128, 128], f32r, tag="sel")

            qt = big_pool.tile([128, 2048], f32r, tag="qt")
            kt = big_pool.tile([128, 2048], f32r, tag="kt")
            # V': [kpos%128, j, head, hd+2]; column 32 is all-ones (denominator
            # trick), column 33 is alignment padding so the ones memset writes
            # 4-byte-aligned element pairs
            va = big_pool.tile([128, 8, 8, 34], bf16, tag="va")
            # O^T blocks: [128, (c 2) x (blk 4) x 512]; blk = (grp, head-pair)
            # valid rows per blk: {0-31} even head, {64-95} odd head; rest 0
            otf = big_pool.tile([128, 4096], f32r, tag="otf")
            # reciprocal staging: rows {32,96}; other rows memset 1.0 (NaN guard)
            # columns: (phase%2) * 1024 + pair * 512
            rcb = big_pool.tile([128, 2048], f32r, tag="rcb")
            # sbuf staging for the broadcast reciprocal (DVE cannot read two
            # psum operands); columns as rcb
            bcs = big_pool.tile([128, 2048], f32, tag="bcs")
            ytile = big_pool.tile([128, 2048], f32, tag="ytile")

            # ---- input DMAs, ordered so the first score matmuls can start
            # after ~1.6us: xt quarters 0,2 and the Q+K weight halves first.
            nc.sync.dma_start(xt[:, 0:512], xt_d[:, 0:512])
            nc.gpsimd.dma_start(xt[:, 1024:1536], xt_d[:, 1024:1536])
            nc.sync.dma_start(qw[:, 0:512], qw_d[:, 0:512])
            nc.gpsimd.dma_start(qw[:, 768:1280], qw_d[:, 768:1280])
            nc.scalar.dma_start(qb[:], qb_d)
            nc.scalar.dma_start(mask[:], mask_d)
            nc.sync.dma_start(xt[:, 512:1024], xt_d[:, 512:1024])
            nc.gpsimd.dma_start(xt[:, 1536:2048], xt_d[:, 1536:2048])
            nc.scalar.dma_start(qw[:, 512:768], qw_d[:, 512:768])
            nc.scalar.dma_start(qw[:, 1280:1536], qw_d[:, 1280:1536])
            nc.sync.dma_start(sel[:], sel_d)
            nc.sync.dma_start(pwx[:], pwx_d)
            nc.sync.dma_start(pb[:], pb_d)

            # warm the Exp activation table (only table used in the kernel);
            # scalar issues no DMAs after this point - it runs only the exps
            warm = const_pool.tile([1, 2], f32, tag="warm")
            nc.scalar.activation(warm[:], mask[0:1, 0:2], AF.Exp, scale=0.0)

            # NaN guards: otf rows outside the valid O rows must be 0.0 (the
            # proj matmul contracts all 128 partitions; 0-weight x NaN = NaN),
            # rcb rows outside {0,64} must be finite for the sel matmul.
            nc.gpsimd.dma_start(va.rearrange("p a b c -> p (a b c)"), vones_d)

            def fill_rcb():
                with nc.allow_low_precision(reason="constant fill"):
                    nc.vector.tensor_scalar(
                        rcb[:], xt[:], 0.0, 1.0,
                        mybir.AluOpType.mult, mybir.AluOpType.add,
                    )

            def fill_otf(half):
                with nc.allow_low_precision(reason="constant fill"):
                    nc.vector.tensor_scalar_mul(
                        otf[:, 2048 * half : 2048 * (half + 1)], xt[:], 0.0
                    )



            # ---- psum tile pre-allocation -------------------------------
            # qk/v projection psums share slots with the attention psum tags;
            # allocation order fixes the slot chains, so all projection tiles
            # must be allocated BEFORE the first phase's accumulators.
            QK_ORDER = [(0, 0), (2, 0), (2, 1), (1, 0), (3, 0), (3, 1), (0, 1), (1, 1)]
            qkps = {}
            for i, (m, cc) in enumerate(QK_ORDER):
                tag = "ov01" if i % 2 == 0 else "ov23"
                qkps[(m, cc)] = ps_ov.tile(
                    [128, 512], f32, tag=tag, name=f"qkps{m}{cc}"
                )
            vps = {}
            for j in range(8):
                tag = "bc" if j % 2 == 0 else "proj"
                vps[j] = ps_bp.tile([128, 512], f32, tag=tag, name=f"vps{j}")

            def qk_tile(m, cc):
                """Q^T/K^T chunk: m 0,1 -> Q halves; 2,3 -> K halves."""
                ps = qkps[(m, cc)]
                for t in range(2):
                    nc.tensor.matmul(
                        ps[:],
                        qw[:, 768 * t + 128 * m : 768 * t + 128 * (m + 1)],
                        xt[:, 1024 * t + 512 * cc : 1024 * t + 512 * (cc + 1)],
                        start=(t == 0),
                        stop=(t == 1),
                    )
                if m < 2:
                    nc.vector.tensor_scalar_add(
                        qt[:, 1024 * m + 512 * cc : 1024 * m + 512 * (cc + 1)],
                        ps[:],
                        qb[:, m : m + 1],
                    )
                else:
                    nc.vector.tensor_copy(
                        kt[:, 1024 * (m - 2) + 512 * cc : 1024 * (m - 2) + 512 * (cc + 1)],
                        ps[:],
                    )

            def v_tile(j):
                """V block j: out [token, chan] bf16 into va cols 1-32."""
                ps = vps[j]
                for t in range(2):
                    nc.tensor.matmul(
                        ps[:, 0:256],
                        xt[:, 1024 * t + 128 * j : 1024 * t + 128 * (j + 1)],
                        qw[:, 768 * t + 512 : 768 * t + 768],
                        start=(t == 0),
                        stop=(t == 1),
                    )
                nc.vector.tensor_copy(
                    va[:, j, :, 0:32],
                    ps[:, 0:256].rearrange("p (h d) -> p h d", d=32),
                )

            # ---- attention phases: (c outer, grp inner) -----------------
            phases = [(0, 0), (0, 1), (1, 0), (1, 1)]  # (c, grp)

            def emit_scores(pidx, j):
                c, grp = phases[pidx]
                s4a = ps_s4a.tile([128, 1024], f32, tag="s4a", name=f"s4a{pidx}{j}")
                s4b = ps_s4b.tile([128, 1024], f32, tag="s4b", name=f"s4b{pidx}{j}")
                for hh in range(4):
                    s4 = s4a if hh < 2 else s4b
                    base = 32 * hh
                    nc.tensor.matmul(
                        s4[:, 512 * (hh % 2) : 512 * (hh % 2 + 1)],
                        kt[base : base + 32,
                           1024 * grp + 128 * j : 1024 * grp + 128 * (j + 1)],
                        qt[base : base + 32,
                           1024 * grp + 512 * c : 1024 * grp + 512 * (c + 1)],
                        start=True,
                        stop=True,
                        tile_position=(base, 0),
                    )
                pta = pt_pool.tile([128, 1024], bf16, tag="pt", name=f"pta{pidx}{j}")
                ptb = pt_pool.tile([128, 1024], bf16, tag="pt", name=f"ptb{pidx}{j}")
                nc.scalar.activation(
                    pta[:], s4a[:], AF.Exp, bias=mask[:, j : j + 1], scale=SCALE
                )
                nc.scalar.activation(
                    ptb[:], s4b[:], AF.Exp, bias=mask[:, j : j + 1], scale=SCALE
                )
                return pta, ptb

            def emit_pv(pidx, j, pta, ptb, ovs):
                c, grp = phases[pidx]
                for hh in range(4):
                    h = 4 * grp + hh
                    ov = ovs[0] if hh < 2 else ovs[1]
                    pt = pta if hh < 2 else ptb
                    rb = 64 * (hh % 2)
                    nc.tensor.matmul(
                        ov[rb : rb + 33, :],
                        va[:, j, h, 0:33],
                        pt[:, 512 * (hh % 2) : 512 * (hh % 2 + 1)],
                        start=(j == 0),
                        stop=(j == 7),
                        tile_position=(0, rb),
                    )

            def emit_norm_half(pidx, pair, ovs):
                """reciprocal of denominators -> broadcast -> normalized O."""
                c, grp = phases[pidx]
                ov = ovs[pair]
                slot = (pidx % 2) * 1024 + pair * 512
                with nc.allow_low_precision(reason="1/denominator, not an accumulation"):
                    nc.vector.reciprocal(
                        rcb[32:33, slot : slot + 512], ov[32:33, :]
                    )
                    nc.vector.reciprocal(
                        rcb[96:97, slot : slot + 512], ov[96:97, :]
                    )
                bc = ps_bp.tile([128, 512], f32, tag="bc", name=f"bc{pidx}{pair}")
                nc.tensor.matmul(
                    bc[:],
                    sel[:],
                    rcb[:, slot : slot + 512],
                    start=True,
                    stop=True,
                )
                nc.vector.tensor_copy(bcs[0:96, slot : slot + 512], bc[0:96, :])
                blk = 2048 * c + 1024 * grp + 512 * pair
                nc.vector.tensor_mul(
                    otf[0:32, blk : blk + 512], ov[0:32, :], bcs[0:32, slot : slot + 512]
                )
                nc.vector.tensor_mul(
                    otf[64:96, blk : blk + 512], ov[64:96, :], bcs[64:96, slot : slot + 512]
                )

            def emit_proj(c):
                for m in range(2):
                    pp = ps_bp.tile([128, 512], f32, tag="proj", name=f"proj{c}{m}")
                    for blk in range(4):
                        nc.tensor.matmul(
                            pp[:],
                            pwx[:, 256 * blk + 128 * m : 256 * blk + 128 * (m + 1)],
                            otf[:, 2048 * c + 512 * blk : 2048 * c + 512 * (blk + 1)],
                            start=(blk == 0),
                            stop=(blk == 3),
                        )
                    nc.vector.tensor_scalar_add(
                        ytile[:, 1024 * m + 512 * c : 1024 * m + 512 * c + 512],
                        pp[:],
                        pb[:, m : m + 1],
                    )
                    (nc.sync if m == 0 else nc.gpsimd).dma_start(
                        yt_d[:, 1024 * m + 512 * c : 1024 * m + 512 * c + 512],
                        ytile[:, 1024 * m + 512 * c : 1024 * m + 512 * c + 512],
                    )

            # first score matmuls' inputs
            qk_tile(0, 0)
            qk_tile(2, 0)

            specials = {
                0: [lambda: v_tile(0), lambda: v_tile(1), lambda: qk_tile(2, 1)],
                1: [lambda: v_tile(2), lambda: v_tile(3), lambda: qk_tile(1, 0)],
                2: [lambda: v_tile(4), lambda: v_tile(5), lambda: qk_tile(3, 0)],
                3: [lambda: v_tile(6), lambda: v_tile(7), lambda: qk_tile(3, 1)],
                4: [lambda: qk_tile(0, 1)],
                5: [lambda: qk_tile(1, 1)],
                6: [fill_rcb],
                7: [lambda: fill_otf(0), lambda: fill_otf(1)],
            }

            steps = [(p, j) for p in range(4) for j in range(8)]
            ovs_by_phase = {}
            pend = None
            deferred = []
            for idx in range(len(steps) + 1):
                if idx < len(steps):
                    p, j = steps[idx]
                    if j == 0:
                        ovs_by_phase[p] = (
                            ps_ov.tile([128, 512], f32, tag="ov01", name=f"ov01_{p}"),
                            ps_ov.tile([128, 512], f32, tag="ov23", name=f"ov23_{p}"),
                        )
                    cur = (p, j, *emit_scores(p, j))
                else:
                    cur = None
                for fn in specials.pop(idx, ()):
                    fn()
                dq, deferred = deferred, []
                for fn in dq:
                    fn()
                if pend is not None:
                    pp_, jj, pta, ptb = pend
                    emit_pv(pp_, jj, pta, ptb, ovs_by_phase[pp_])
                    if jj == 7:
                        cph, cgrp = phases[pp_]
                        if idx < len(steps):
                            emit_norm_half(pp_, 0, ovs_by_phase[pp_])
                            deferred.append(
                                lambda pp_=pp_: emit_norm_half(pp_, 1, ovs_by_phase[pp_])
                            )
                            if cgrp == 1:
                                deferred.append(lambda cph=cph: emit_proj(cph))
                        else:
                            emit_norm_half(pp_, 0, ovs_by_phase[pp_])
                            emit_norm_half(pp_, 1, ovs_by_phase[pp_])
                            emit_proj(cph)
                pend = cur

    return nc


def _get_nc():
    if "nc" not in _cached:
        _cached["nc"] = _build_nc()
    return _cached["nc"]


def _pack_per_partition(a2d):
    """[2*128, F] -> [128, 2*F] with tile t at cols F*t."""
    n, f = a2d.shape
    t = n // 128
    return np.ascontiguousarray(
        a2d.reshape(t, 128, f).transpose(1, 0, 2).reshape(128, t * f)
    )


def _prepare(carrier_tokens, ct_mask, batch_num_windows, qkv_w, qkv_b, proj_w, proj_b):
    """Host-side bookkeeping: ragged->padded scatter, weight packing.
    Returns (in_maps, ctx) where ctx carries what postprocessing needs."""
    carrier_tokens = np.asarray(carrier_tokens, dtype=np.float32)
    ct_mask = np.asarray(ct_mask, dtype=np.float32)
    lens = np.asarray(batch_num_windows).astype(np.int64)
    qkv_w = np.asarray(qkv_w, dtype=np.float32)
    qkv_b = np.asarray(qkv_b, dtype=np.float32)
    proj_w = np.asarray(proj_w, dtype=np.float32)
    proj_b = np.asarray(proj_b, dtype=np.float32)

    total = carrier_tokens.shape[0]

    # ragged -> padded bookkeeping (mirrors the reference's scatter semantics:
    # OOB scatter indices dropped, OOB gather indices clipped)
    offsets = np.concatenate([[0], np.cumsum(lens)])
    tok = np.arange(total)
    b_id = np.searchsorted(offsets[1:], tok, side="right")
    w_id = tok - offsets[np.minimum(b_id, B)]
    flat_idx = b_id * MAXW + w_id
    valid = flat_idx < B * MAXW
    padded = np.zeros((B * MAXW, C), np.float32)
    padded[flat_idx[valid]] = carrier_tokens[valid]
    padded = padded.reshape(B, MAXW, C)

    mask_col = np.ascontiguousarray(ct_mask[:, 0, :])  # [B, MAXW]

    # host-side exact weight transforms
    pb_eff = qkv_b[2 * C : 3 * C] @ proj_w + proj_b

    qw_packed = _pack_per_partition(qkv_w)                      # [128, 1536]
    qb_packed = np.ascontiguousarray(qkv_b[0:C].reshape(2, 128).T)
    pb_packed = np.ascontiguousarray(pb_eff.reshape(2, 128).T)

    # pwx: [128, blk(4) x m(2) x 128], rows {0-31} even head of blk, rows
    # {64-95} odd head of blk, other rows zero (matching the otf layout)
    pwx = np.zeros((128, 1024), np.float32)
    for blk in range(4):
        for m in range(2):
            colbase = 256 * blk + 128 * m
            for half, rowbase in ((0, 0), (1, 64)):
                h = 2 * blk + half
                # rows rowbase..rowbase+31 <- proj_w[32h..32h+31, 128m..]
                pwx[rowbase : rowbase + 32, colbase : colbase + 128] = proj_w[
                    32 * h : 32 * h + 32, 128 * m : 128 * (m + 1)
                ]

    sel_arr = np.zeros((128, 128), np.float32)
    sel_arr[32, 0:32] = 1.0
    sel_arr[96, 64:96] = 1.0

    import ml_dtypes
    vones_arr = np.ones((128, 2176), ml_dtypes.bfloat16)

    in_maps = []
    for b in range(B):
        xt = _pack_per_partition(padded[b].T)                   # [128, 2048]
        mb = np.ascontiguousarray(mask_col[b].reshape(8, 128).T)
        in_maps.append(
            {
                "xt": xt,
                "qw": qw_packed,
                "qb": qb_packed,
                "pwx": pwx,
                "pb": pb_packed,
                "mask": mb,
                "sel": sel_arr,
                "vones": vones_arr,
            }
        )

    ctx = {
        "flat_idx": flat_idx,
        "mask_col": mask_col,
        "padded": padded,
        "qkv_w": qkv_w,
        "qkv_b": qkv_b,
        "proj_w": proj_w,
        "proj_b": proj_b,
    }
    return in_maps, ctx


def _postprocess(results, ctx):
    """Per-core outputs -> full ragged output (gather + degenerate-row fix)."""
    flat_idx = ctx["flat_idx"]
    mask_col = ctx["mask_col"]
    padded = ctx["padded"]
    qkv_w, qkv_b = ctx["qkv_w"], ctx["qkv_b"]
    proj_w, proj_b = ctx["proj_w"], ctx["proj_b"]

    y_pad = np.empty((B, MAXW, C), np.float32)
    for b in range(B):
        yt = results[b]["yt"]                                   # [128, 2048]
        y_t = yt.reshape(128, 2, MAXW).transpose(1, 0, 2).reshape(C, MAXW)
        y_pad[b] = y_t.T
    y_flat = y_pad.reshape(B * MAXW, C)
    gather_idx = np.clip(flat_idx, 0, B * MAXW - 1)
    out = y_flat[gather_idx]

    # degenerate rows: gathered positions whose key mask is fully masked.
    # The reference's softmax (with max-subtraction) gives uniform weights
    # there; our exp underflows to 0/0. Recompute those rows exactly.
    row_b = np.minimum(gather_idx // MAXW, B - 1)
    degenerate_batches = [b for b in range(B) if np.all(mask_col[b] < NEG_THRESH)]
    for b in degenerate_batches:
        rows = np.nonzero(row_b == b)[0]
        if rows.size == 0:
            continue
        vmat = padded[b] @ qkv_w[:, 2 * C : 3 * C] + qkv_b[2 * C : 3 * C]
        mean_v = vmat.mean(axis=0)  # uniform attention, same for all heads
        fix = mean_v @ proj_w + proj_b
        out[rows] = fix.astype(np.float32)

    return np.ascontiguousarray(out.astype(np.float32))


def run_device(in_maps, **spmd_kwargs):
    from concourse import bass_utils

    nc = _get_nc()
    return bass_utils.run_bass_kernel_spmd(
        nc, in_maps, core_ids=list(range(B)), **spmd_kwargs
    )


def kernel(carrier_tokens, ct_mask, batch_num_windows, qkv_w, qkv_b, proj_w, proj_b):
    in_maps, ctx = _prepare(
        carrier_tokens, ct_mask, batch_num_windows, qkv_w, qkv_b, proj_w, proj_b
    )
    res = run_device(in_maps, trace=False)
    return _postprocess(res.results, ctx)
